# revision 10
# baseline (speedup 1.0000x reference)
"""Causal attention kernel for Trainium2 (Bass/Tile), 8-core SPMD.

Problem: out = softmax(causal(C @ B^T / sqrt(64))) @ x
  x, B, C: [2, 8, 4096, 64] fp32  (V, K, Q respectively)

Sharding: batch*heads = 16 slices -> 2 per core across 8 cores.
Each core runs causal attention for its 2 head-slices independently.

Per-head algorithm (L=4096, D=64, 128x128 score tiles, i=query tile,
j=key tile, causal j<=i):
  - Pre-transpose Q and K to [64, L] layout via PE transposes (contraction
    dim d must live on partitions for both matmul operands).
  - V is augmented with a ones-column -> V1 [128, 65] per j-tile; the PV
    matmul then also accumulates the softmax denominator (row 64).
  - Scores are computed TRANSPOSED: S^T[kk, q] = K_j @ Q^T slice, so the
    exp output feeds the PV matmul directly as the moving operand --
    no per-tile transpose of the probabilities is needed.
  - exp on ScalarE with scale=1/8 fused; no max-subtraction (scores are
    ~N(0,1); max over 8M samples < ~7, exp(7) is tiny vs fp32 range).
  - Causal masking inside the diagonal tile: multiply exp by a 0/1
    upper-triangular mask (valid kk <= q).
  - i-range processed in halves of 16 tiles (2048 q): 4 PSUM banks hold
    the O^T accumulators [65, 512], 2x2 banks ping-pong the score tiles.
  - Epilogue per output bank: copy to SBUF, PE-transpose each [65,128]
    to [128,65], divide by the denominator column, DMA out.
"""

import os
from contextlib import ExitStack

import numpy as np

L = 4096
D = 64
P = 128
NT = L // P           # 32 query/key tiles per head
HALF_T = NT // 2      # i-tiles per half-pass
HEADS_PER_CORE = 2
N_CORES = 8

_cache = {}


def _build_nc():
    import concourse.mybir as mybir
    import concourse.tile as tile
    from concourse import bacc
    from concourse.masks import make_identity

    f32 = mybir.dt.float32
    FR = mybir.dt.float32r
    bf16 = mybir.dt.bfloat16
    EXP = mybir.ActivationFunctionType.Exp

    nc = bacc.Bacc("TRN2", target_bir_lowering=False, debug=False)

    x_t = nc.dram_tensor("x", (HEADS_PER_CORE, L, D), FR, kind="ExternalInput")
    b_t = nc.dram_tensor("B", (HEADS_PER_CORE, L, D), f32, kind="ExternalInput")
    c_t = nc.dram_tensor("C", (HEADS_PER_CORE, L, D), f32, kind="ExternalInput")
    o_t = nc.dram_tensor("out", (HEADS_PER_CORE, L, D), f32, kind="ExternalOutput")
    x_ap, b_ap, c_ap, o_ap = x_t.ap(), b_t.ap(), c_t.ap(), o_t.ap()

    with tile.TileContext(nc) as tc, ExitStack() as ctx:
        const = ctx.enter_context(tc.tile_pool(name="const", bufs=1))
        identity = const.tile([P, P], f32)
        make_identity(nc, identity[:])
        # Diagonal-tile mask in S^T coords [kk, q]: keep kk <= q.
        dmask = const.tile([P, P], f32)
        nc.gpsimd.memset(dmask[:], 1.0)
        nc.gpsimd.affine_select(
            out=dmask[:],
            in_=dmask[:],
            compare_op=mybir.AluOpType.is_ge,
            fill=0.0,
            base=0,
            pattern=[[1, P]],       # +q
            channel_multiplier=-1,  # -kk  => keep where q - kk >= 0
        )

        ones32 = const.tile([P, NT], f32, name="ones32")
        nc.vector.memset(ones32[:], 1.0)

        qkv = ctx.enter_context(tc.tile_pool(name="qkv", bufs=1))
        qt = {}
        kt = {}
        v1 = {}

        # ---- Stage inputs: V1 (with ones col), Q^T, K^T for both heads ----
        with (
            tc.tile_pool(name="stage", bufs=3) as stage,
            tc.tile_pool(name="tpsum", bufs=2, space="PSUM") as tpsum,
        ):
            for h in range(HEADS_PER_CORE):
                v1[h] = qkv.tile([P, NT, D + 1], FR, name=f"v1_{h}", tag=f"v1_{h}")
                vstage = stage.tile([P, NT, D], FR, name="vstage", tag="vstage")
                nc.sync.dma_start(
                    out=vstage[:],
                    in_=x_ap[h].rearrange("(j p) d -> p j d", p=P),
                )
                nc.vector.tensor_copy(v1[h][:, :, 0:D], vstage[:])
                nc.vector.tensor_copy(v1[h][:, :, D], ones32[:])

                for nm, src, dst_map in (("q", c_ap, qt), ("k", b_ap, kt)):
                    dst = qkv.tile([D, NT, P], FR, name=f"{nm}t_{h}", tag=f"{nm}t_{h}")
                    dst_map[h] = dst
                    for g in range(NT // 4):  # 4 i-tiles per transpose batch
                        st = stage.tile([P, 4, D], f32, name="st", tag="stage_in")
                        nc.sync.dma_start(
                            out=st[:],
                            in_=src[h, g * 4 * P : (g + 1) * 4 * P].rearrange(
                                "(a p) d -> p a d", p=P
                            ),
                        )
                        pt = tpsum.tile([D, 4, P], f32, name="pt", tag="tp")
                        for a in range(4):
                            nc.tensor.transpose(
                                pt[:, a], st[:, a], identity[:]
                            )
                        nc.vector.tensor_copy(dst[:, g * 4 : (g + 1) * 4], pt[:])

        # ---- Attention ----
        with (
            tc.tile_pool(name="score", bufs=2, space="PSUM") as score_pool,
            tc.tile_pool(name="oacc", bufs=4, space="PSUM") as oacc_pool,
            tc.tile_pool(name="exps", bufs=3) as exps_pool,
            tc.tile_pool(name="epi", bufs=3) as epi_pool,
        ):
            for h in range(HEADS_PER_CORE):
                for half in range(2):
                    hi0 = half * HALF_T  # first i-tile of this half
                    obank = [
                        oacc_pool.tile([D + 1, 512], f32, name="obank", tag="oacc")
                        for _ in range(4)
                    ]
                    for j in range(hi0 + HALF_T):
                        jt = max(j, hi0)          # first valid i-tile
                        q0l = (jt - hi0) * P      # local q offset in [0, 2048)
                        kslice = kt[h][:, j]                   # [64, 128]
                        vslice = v1[h][:, j]                   # [128, 65]

                        for g in range(q0l // 1024, 2):
                            gs = g * 1024
                            vs = max(q0l, gs)     # valid start (local)
                            sc = score_pool.tile([P, 1024], f32, name="sc", tag="score")
                            et = exps_pool.tile([P, 1024], FR, name="et", tag="exps")

                            # QK matmuls on the 512 grid within this group
                            for b2 in range(2):
                                bs = gs + b2 * 512
                                cs = max(vs, bs)
                                w = bs + 512 - cs
                                if w <= 0:
                                    continue
                                t0 = hi0 + cs // P   # first i-tile of chunk
                                nc.tensor.matmul(
                                    sc[:, cs - gs : cs - gs + w],
                                    lhsT=kslice,
                                    rhs=qt[h][:, t0 : t0 + w // P],
                                    start=True,
                                    stop=True,
                                )

                            # exp(score/8): PSUM -> SBUF
                            nc.scalar.activation(
                                et[:, vs - gs :], sc[:, vs - gs :], EXP,
                                scale=0.125,
                            )

                            # causal mask on the diagonal tile
                            if j >= hi0 and gs <= q0l < gs + 1024:
                                off = q0l - gs
                                nc.vector.tensor_mul(
                                    et[:, off : off + P],
                                    et[:, off : off + P],
                                    dmask[:],
                                )

                            # PV matmuls, accumulating O^T + denominator
                            for b2 in range(2):
                                bs = gs + b2 * 512
                                cs = max(vs, bs)
                                w = bs + 512 - cs
                                if w <= 0:
                                    continue
                                bank = g * 2 + b2
                                nc.tensor.matmul(
                                    obank[bank][:, cs - bs : cs - bs + w],
                                    lhsT=vslice,
                                    rhs=et[:, cs - gs : cs - gs + w],
                                    start=(j == 0),
                                    stop=(j == hi0 + 4 * bank + 3),
                                )

                    # ---- Epilogue: transpose O^T, divide by denom, store ----
                    for bank in range(4):
                        osb = epi_pool.tile([D + 1, 512], f32, name="osb", tag="osb")
                        nc.vector.tensor_copy(osb[:], obank[bank][:])
                        for a in range(4):
                            it = hi0 + 4 * bank + a
                            tpt = score_pool.tile([P, D + 1], f32, name="tpt", tag="score")
                            nc.tensor.transpose(
                                tpt[:],
                                osb[:, a * P : (a + 1) * P],
                                identity[: D + 1, : D + 1],
                            )
                            rec = epi_pool.tile([P, 1], f32, name="rec", tag="rec")
                            nc.vector.reciprocal(rec[:], tpt[:, D : D + 1])
                            ot = epi_pool.tile([P, D], f32, name="ot", tag="ot")
                            nc.vector.tensor_scalar_mul(
                                ot[:], tpt[:, 0:D], rec[:]
                            )
                            nc.sync.dma_start(
                                out=o_ap[h, it * P : (it + 1) * P],
                                in_=ot[:],
                            )

    nc.compile()
    return nc


def _get_nc():
    if "nc" not in _cache:
        _cache["nc"] = _build_nc()
    return _cache["nc"]


def kernel(x: np.ndarray, B: np.ndarray, C: np.ndarray) -> np.ndarray:
    from concourse import bass_utils

    BATCH, H = x.shape[0], x.shape[1]
    nbh = BATCH * H
    xf = np.ascontiguousarray(x.reshape(nbh, L, D), dtype=np.float32)
    bf = np.ascontiguousarray(B.reshape(nbh, L, D), dtype=np.float32)
    cf = np.ascontiguousarray(C.reshape(nbh, L, D), dtype=np.float32)

    nc = _get_nc()
    in_maps = []
    for c in range(N_CORES):
        s = slice(c * HEADS_PER_CORE, (c + 1) * HEADS_PER_CORE)
        in_maps.append(
            {
                "x": np.ascontiguousarray(xf[s]),
                "B": np.ascontiguousarray(bf[s]),
                "C": np.ascontiguousarray(cf[s]),
            }
        )

    trace = bool(int(os.environ.get("KERNEL_TRACE", "0")))
    res = bass_utils.run_bass_kernel_spmd(
        nc,
        in_maps,
        core_ids=list(range(N_CORES)),
        trace=trace,
        trace_cores=list(range(N_CORES)) if trace else None,
    )
    _cache["last_result"] = res

    out = np.empty((nbh, L, D), dtype=np.float32)
    for c in range(N_CORES):
        out[c * HEADS_PER_CORE : (c + 1) * HEADS_PER_CORE] = res.results[c]["out"]
    return out.reshape(BATCH, H, L, D)
